# revision 4
# baseline (speedup 1.0000x reference)
"""DCT-compressed attention (nn_DCTAttentionIdeal) on 8 Trainium2 NeuronCores.

Math (per head, reference ordering):
    S    = (Q*s) @ (K*mask*s)^T with s = D**-0.25             [N,N]
    atn  = softmax(S, axis=-1)
    Vd   = Qd @ (V*mask)                                      [M,D]
    out  = Qd^T @ ((Qd @ atn @ Qd^T) @ Vd)                    [N,D]

Kernel reshaping (exact in real arithmetic):
  - softmax max-subtraction skipped (scores ~N(0,1) after the 1/8 scale,
    which is folded into the Exp activation's `scale`).
  - per-row 1/denom folded into DCT columns:
        A1^T[k,m] = sum_q exp(S)[q,k] * (Qd^T[q,m]/denom[q])
    so the [N,N] exp matrix is consumed unnormalized straight from SBUF.
  - final contraction reassociated: out = Qd^T @ (G @ Vd), G = A1 @ Qd^T.

dtypes: the two O(N^2 M) matmuls (scores' exp -> A1^T) run bf16 (exp storage);
everything else runs float32r (tf32-class precision, bf16-class speed).
Phase A (exp on ScalarE) of one q-group overlaps phase B (A1^T on TensorE)
of the previous group via a 2-group software pipeline; GT accumulates both
groups' partial A1^T tiles so no extra adds are needed.

Sharding: batch*heads (2*16=32) split 4-per-core across 8 cores; Q_dct
replicated; no cross-core communication.  Host pre-transposes Q and Q_dct
(pure layout); masking, K transpose, softmax and all DCT algebra run
on-device.
"""

import numpy as np
import ml_dtypes

import concourse.bass as bass
import concourse.tile as tile
from concourse import bacc, mybir
from concourse import bass_utils
from concourse.masks import make_identity

F32 = mybir.dt.float32
BF16 = mybir.dt.bfloat16
F32R = mybir.dt.float32r
NPBF16 = ml_dtypes.bfloat16
AF = mybir.ActivationFunctionType
ALU = mybir.AluOpType
AX = mybir.AxisListType

B, H, N, D, M = 2, 16, 2048, 64, 256
NCORES = 8
HPC = (B * H) // NCORES  # heads per core = 4
NT = N // 128            # 16 (q and k 128-blocks)
MT = M // 128            # 2
NQG = 2                  # q-group count (software pipeline A||B)


def _emit(tc, ctx, io):
    nc = tc.nc
    P = 128
    GQ = NT // NQG               # q-blocks per group
    SCH = min(1024, N)           # score chunk (elements) per activation
    NCH = N // SCH               # activations per q-block

    sh = ctx.enter_context(tc.tile_pool(name="shared", bufs=1))
    exp_pool = ctx.enter_context(tc.tile_pool(name="exp", bufs=GQ + 1))
    kt_pool = ctx.enter_context(tc.tile_pool(name="ktr", bufs=2))
    qt_pool = ctx.enter_context(tc.tile_pool(name="qtr", bufs=2))
    vl_pool = ctx.enter_context(tc.tile_pool(name="vload", bufs=2))
    vm_pool = ctx.enter_context(tc.tile_pool(name="vmask", bufs=2))
    cq_pool = ctx.enter_context(tc.tile_pool(name="cq", bufs=2))
    a1_pool = ctx.enter_context(tc.tile_pool(name="a1t", bufs=2))
    gt_pool = ctx.enter_context(tc.tile_pool(name="gt", bufs=2))
    vd_pool = ctx.enter_context(tc.tile_pool(name="vd", bufs=2))
    y_pool = ctx.enter_context(tc.tile_pool(name="y", bufs=2))
    ost_pool = ctx.enter_context(tc.tile_pool(name="ost", bufs=4))
    kld_pool = ctx.enter_context(tc.tile_pool(name="kld", bufs=4))
    msk_pool = ctx.enter_context(tc.tile_pool(name="msk", bufs=2))
    st_pool = ctx.enter_context(tc.tile_pool(name="stats", bufs=6))

    ps_s = ctx.enter_context(tc.tile_pool(name="ps_s", bufs=2, space="PSUM"))
    ps_a1 = ctx.enter_context(tc.tile_pool(name="ps_a1", bufs=2, space="PSUM"))
    ps_gt = ctx.enter_context(tc.tile_pool(name="ps_gt", bufs=1, space="PSUM"))
    ps_m = ctx.enter_context(tc.tile_pool(name="ps_m", bufs=1, space="PSUM"))

    # --- shared, once per core ------------------------------------------
    ident = sh.tile([P, P], F32)
    make_identity(nc, ident[:])

    qdt32 = sh.tile([P, NT, M], F32)    # Qd^T [k,m] (CqT source)
    nc.sync.dma_start(qdt32[:], io["QdT32"].rearrange("(t p) m -> p t m", p=P))
    qdtr = sh.tile([P, NT, M], F32R)    # Qd^T (Vd lhsT)
    nc.sync.dma_start(qdtr[:], io["QdTr"].rearrange("(t p) m -> p t m", p=P))
    qdt16 = sh.tile([P, NT, M], BF16)   # Qd^T (GT lhsT)
    nc.sync.dma_start(qdt16[:], io["QdT16"].rearrange("(t p) m -> p t m", p=P))
    qdnr = sh.tile([P, MT, N], F32R)    # Qd [m,q] (out lhsT)
    nc.sync.dma_start(qdnr[:], io["QdNr"].rearrange("(c p) q -> p c q", p=P))

    for h in range(HPC):
        # --- per-head input prep ---------------------------------------
        mk = msk_pool.tile([P, NT], F32)
        nc.sync.dma_start(mk[:], io["maskT"][h])

        qt = qt_pool.tile([64, N], F32R)           # Q^T [d,q]
        nc.sync.dma_start(qt[:], io["QT"][h])

        vl = vl_pool.tile([P, NT, D], F32)         # V natural [k,d]
        nc.sync.dma_start(vl[:], io["V"][h].rearrange("(t p) d -> p t d", p=P))
        vm = vm_pool.tile([P, NT, D], F32R)        # (V*mask) rounded to f32r
        for t in range(NT):
            nc.vector.tensor_scalar_mul(vm[:, t, :], vl[:, t, :], mk[:, t : t + 1])

        kts = kt_pool.tile([64, N], F32R)          # (K*mask)^T [d,k]
        k_r = io["K"][h].rearrange("(t p) d -> t p d", p=P)
        for t in range(NT):
            kl = kld_pool.tile([P, D], F32, tag="kld")
            nc.sync.dma_start(kl[:], k_r[t])
            nc.vector.tensor_scalar_mul(kl[:], kl[:], mk[:, t : t + 1])
            pt = ps_m.tile([D, P], F32, tag="misc")
            nc.tensor.transpose(pt[:], kl[:], ident[:])
            nc.vector.tensor_copy(kts[:, t * P : (t + 1) * P], pt[:])

        # --- Vd = Qd @ (V*m)  -> [M, D] --------------------------------
        vd = vd_pool.tile([P, MT, D], F32R)
        for mh in range(MT):
            vps = ps_m.tile([P, D], F32, tag="misc")
            for t in range(NT):
                nc.tensor.matmul(
                    vps[:],
                    lhsT=qdtr[:, t, mh * P : (mh + 1) * P],
                    rhs=vm[:, t, :],
                    start=(t == 0),
                    stop=(t == NT - 1),
                )
            nc.vector.tensor_copy(vd[:, mh, :], vps[:])

        # --- phases A (scores->exp->CqT) and B (A1^T), group-pipelined --
        cq = cq_pool.tile([P, NT, M], BF16)
        a1 = a1_pool.tile([P, NT, NQG, M], BF16)
        exps = {}
        for g in range(NQG):
            # phase A for group g
            for q in range(g * GQ, (g + 1) * GQ):
                ex = exp_pool.tile([P, N], BF16, tag="exp")
                sums = st_pool.tile([P, NCH], F32, tag="sums")
                for c in range(NCH):
                    sps = ps_s.tile([P, SCH], F32, tag="s")
                    for j in range(SCH // 512):
                        nc.tensor.matmul(
                            sps[:, j * 512 : (j + 1) * 512],
                            lhsT=qt[:, q * P : (q + 1) * P],
                            rhs=kts[:, c * SCH + j * 512 : c * SCH + (j + 1) * 512],
                            start=True,
                            stop=True,
                        )
                    nc.scalar.activation(
                        ex[:, c * SCH : (c + 1) * SCH],
                        sps[:],
                        AF.Exp,
                        scale=0.125,
                        accum_out=sums[:, c : c + 1],
                    )
                den = st_pool.tile([P, 1], F32, tag="den")
                if NCH > 1:
                    nc.vector.tensor_reduce(den[:], sums[:], axis=AX.X, op=ALU.add)
                else:
                    den = sums
                rec = st_pool.tile([P, 1], F32, tag="rec")
                nc.vector.reciprocal(rec[:], den[:])
                nc.vector.tensor_scalar_mul(cq[:, q, :], qdt32[:, q, :], rec[:])
                exps[q] = ex

            # phase B for group g: A1^T partial over this group's q-blocks
            for kc in range(NT):
                aps_ = ps_a1.tile([P, M], F32, tag="a1")
                for qi in range(GQ):
                    q = g * GQ + qi
                    nc.tensor.matmul(
                        aps_[:],
                        lhsT=exps[q][:, kc * P : (kc + 1) * P],
                        rhs=cq[:, q, :],
                        start=(qi == 0),
                        stop=(qi == GQ - 1),
                    )
                nc.vector.tensor_copy(a1[:, kc, g, :], aps_[:])

        # --- G^T[n,m] = sum_k QdT[k,n] * A1T[k,m] (both group partials) -
        gt = gt_pool.tile([P, MT, M], F32R)
        gps = ps_gt.tile([P, MT * M], F32, tag="g")
        for nh in range(MT):
            for kc in range(NT):
                for g in range(NQG):
                    nc.tensor.matmul(
                        gps[:, nh * M : (nh + 1) * M],
                        lhsT=qdt16[:, kc, nh * P : (nh + 1) * P],
                        rhs=a1[:, kc, g, :],
                        start=(kc == 0 and g == 0),
                        stop=(kc == NT - 1 and g == NQG - 1),
                    )
            nc.vector.tensor_copy(gt[:, nh, :], gps[:, nh * M : (nh + 1) * M])

        # --- y[m,d] = sum_n GT[n,m] * Vd[n,d] ---------------------------
        yt = y_pool.tile([P, MT, D], F32R)
        for mh in range(MT):
            yps = ps_m.tile([P, D], F32, tag="misc")
            for nh in range(MT):
                nc.tensor.matmul(
                    yps[:],
                    lhsT=gt[:, nh, mh * P : (mh + 1) * P],
                    rhs=vd[:, nh, :],
                    start=(nh == 0),
                    stop=(nh == MT - 1),
                )
            nc.vector.tensor_copy(yt[:, mh, :], yps[:])

        # --- out[q,d] = sum_m Qd[m,q] * y[m,d] --------------------------
        o_r = io["out"][h].rearrange("(t p) d -> t p d", p=P)
        for q in range(NT):
            ops_ = ps_m.tile([P, D], F32, tag="misc")
            for mh in range(MT):
                nc.tensor.matmul(
                    ops_[:],
                    lhsT=qdnr[:, mh, q * P : (q + 1) * P],
                    rhs=yt[:, mh, :],
                    start=(mh == 0),
                    stop=(mh == MT - 1),
                )
            ost = ost_pool.tile([P, D], F32, tag="ost")
            nc.vector.tensor_copy(ost[:], ops_[:])
            nc.sync.dma_start(o_r[q], ost[:])


def build_nc():
    from contextlib import ExitStack

    nc = bacc.Bacc("TRN2", target_bir_lowering=False, debug=False)
    io = {
        "QT": nc.dram_tensor("QT", [HPC, 64, N], F32R, kind="ExternalInput").ap(),
        "K": nc.dram_tensor("K", [HPC, N, D], F32, kind="ExternalInput").ap(),
        "V": nc.dram_tensor("V", [HPC, N, D], F32, kind="ExternalInput").ap(),
        "maskT": nc.dram_tensor("maskT", [HPC, 128, NT], F32, kind="ExternalInput").ap(),
        "QdT32": nc.dram_tensor("QdT32", [N, M], F32, kind="ExternalInput").ap(),
        "QdTr": nc.dram_tensor("QdTr", [N, M], F32R, kind="ExternalInput").ap(),
        "QdT16": nc.dram_tensor("QdT16", [N, M], BF16, kind="ExternalInput").ap(),
        "QdNr": nc.dram_tensor("QdNr", [M, N], F32R, kind="ExternalInput").ap(),
        "out": nc.dram_tensor("out", [HPC, N, D], F32, kind="ExternalOutput").ap(),
    }
    with tile.TileContext(nc) as tc:
        with ExitStack() as ctx:
            _emit(tc, ctx, io)
    nc.compile()
    return nc


_NC = None


def _get_nc():
    global _NC
    if _NC is None:
        _NC = build_nc()
    return _NC


def make_in_maps(Q, K, V, mask, Q_dct):
    Q = np.asarray(Q, dtype=np.float32).reshape(B * H, N, D)
    K = np.asarray(K, dtype=np.float32).reshape(B * H, N, D)
    V = np.asarray(V, dtype=np.float32).reshape(B * H, N, D)
    mask = np.asarray(mask, dtype=np.float32)
    Q_dct = np.asarray(Q_dct, dtype=np.float32)

    QT = np.ascontiguousarray(Q.transpose(0, 2, 1))
    QdT = np.ascontiguousarray(Q_dct.T)
    QdT16 = QdT.astype(NPBF16)
    QdN = np.ascontiguousarray(Q_dct)
    # maskT[b, p, t] = mask[b, t*128 + p]
    maskT = np.ascontiguousarray(mask.reshape(B, NT, 128).transpose(0, 2, 1))

    in_maps = []
    for c in range(NCORES):
        sl = slice(HPC * c, HPC * (c + 1))
        heads = range(HPC * c, HPC * (c + 1))
        in_maps.append(
            {
                "QT": np.ascontiguousarray(QT[sl]),
                "K": np.ascontiguousarray(K[sl]),
                "V": np.ascontiguousarray(V[sl]),
                "maskT": np.ascontiguousarray(
                    np.stack([maskT[hp // H] for hp in heads])
                ),
                "QdT32": QdT,
                "QdTr": QdT,
                "QdT16": QdT16,
                "QdNr": QdN,
            }
        )
    return in_maps


def run_on_device(in_maps, **kwargs):
    nc = _get_nc()
    return bass_utils.run_bass_kernel_spmd(
        nc, in_maps, core_ids=list(range(NCORES)), **kwargs
    )


def kernel(Q, K, V, mask, Q_dct):
    in_maps = make_in_maps(Q, K, V, mask, Q_dct)
    res = run_on_device(in_maps)
    out = np.empty((B * H, N, D), dtype=np.float32)
    for c in range(NCORES):
        out[HPC * c : HPC * (c + 1)] = res.results[c]["out"]
    return out.reshape(B, H, N, D)


# revision 5
# speedup vs baseline: 1.2289x; 1.2289x over previous
"""DCT-compressed attention (nn_DCTAttentionIdeal) on 8 Trainium2 NeuronCores.

Math (per head, reference ordering):
    S    = (Q*s) @ (K*mask*s)^T with s = D**-0.25             [N,N]
    atn  = softmax(S, axis=-1)
    Vd   = Qd @ (V*mask)                                      [M,D]
    out  = Qd^T @ ((Qd @ atn @ Qd^T) @ Vd)                    [N,D]

Kernel reshaping (exact in real arithmetic):
  - softmax max-subtraction skipped (scores ~N(0,1) after the 1/8 scale,
    which is folded into the Exp activation's `scale`).
  - per-row 1/denom folded into DCT columns:
        A1^T[k,m] = sum_q exp(S)[q,k] * (Qd^T[q,m]/denom[q])
    so the [N,N] exp matrix is consumed unnormalized straight from SBUF.
  - final contraction reassociated: out = Qd^T @ (G @ Vd), G = A1 @ Qd^T.

dtypes: the two O(N^2 M) matmuls (scores' exp -> A1^T) run bf16 (exp storage);
everything else runs float32r (tf32-class precision, bf16-class speed).
Phase A (exp on ScalarE) of one q-group overlaps phase B (A1^T on TensorE)
of the previous group via a 2-group software pipeline; GT accumulates both
groups' partial A1^T tiles so no extra adds are needed.

Sharding: batch*heads (2*16=32) split 4-per-core across 8 cores; Q_dct
replicated; no cross-core communication.  Host pre-transposes Q and Q_dct
(pure layout); masking, K transpose, softmax and all DCT algebra run
on-device.
"""

import numpy as np
import ml_dtypes

import concourse.bass as bass
import concourse.tile as tile
from concourse import bacc, mybir
from concourse import bass_utils
from concourse.masks import make_identity

F32 = mybir.dt.float32
BF16 = mybir.dt.bfloat16
F32R = mybir.dt.float32r
NPBF16 = ml_dtypes.bfloat16
AF = mybir.ActivationFunctionType
ALU = mybir.AluOpType
AX = mybir.AxisListType

B, H, N, D, M = 2, 16, 2048, 64, 256
NCORES = 8
HPC = (B * H) // NCORES  # heads per core = 4
NT = N // 128            # 16 (q and k 128-blocks)
MT = M // 128            # 2
NQG = 2                  # q-group count (software pipeline A||B)


def _emit(tc, ctx, io):
    nc = tc.nc
    P = 128
    GQ = NT // NQG               # q-blocks per group
    SCH = min(1024, N)           # score chunk (elements) per activation
    NCH = N // SCH               # activations per q-block

    sh = ctx.enter_context(tc.tile_pool(name="shared", bufs=1))
    exp_pool = ctx.enter_context(tc.tile_pool(name="exp", bufs=GQ + 4))
    kl_pool = ctx.enter_context(tc.tile_pool(name="kload", bufs=1))
    kt_pool = ctx.enter_context(tc.tile_pool(name="ktr", bufs=2))
    qt_pool = ctx.enter_context(tc.tile_pool(name="qtr", bufs=2))
    vl_pool = ctx.enter_context(tc.tile_pool(name="vload", bufs=1))
    vm_pool = ctx.enter_context(tc.tile_pool(name="vmask", bufs=2))
    cq_pool = ctx.enter_context(tc.tile_pool(name="cq", bufs=2))
    a1_pool = ctx.enter_context(tc.tile_pool(name="a1t", bufs=2))
    gt_pool = ctx.enter_context(tc.tile_pool(name="gt", bufs=2))
    vd_pool = ctx.enter_context(tc.tile_pool(name="vd", bufs=2))
    y_pool = ctx.enter_context(tc.tile_pool(name="y", bufs=2))
    ost_pool = ctx.enter_context(tc.tile_pool(name="ost", bufs=4))
    kld_pool = ctx.enter_context(tc.tile_pool(name="kld", bufs=4))
    msk_pool = ctx.enter_context(tc.tile_pool(name="msk", bufs=2))
    st_pool = ctx.enter_context(tc.tile_pool(name="stats", bufs=6))
    kld_pool = None  # removed: K arrives pre-transposed from host

    ps_s = ctx.enter_context(tc.tile_pool(name="ps_s", bufs=2, space="PSUM"))
    ps_a1 = ctx.enter_context(tc.tile_pool(name="ps_a1", bufs=2, space="PSUM"))
    ps_gt = ctx.enter_context(tc.tile_pool(name="ps_gt", bufs=1, space="PSUM"))
    ps_m = ctx.enter_context(tc.tile_pool(name="ps_m", bufs=1, space="PSUM"))

    # --- shared, once per core ------------------------------------------
    maskB = sh.tile([64, N], F32)       # mask row broadcast over d-partitions
    nc.sync.dma_start(maskB[:], io["maskB"])

    qdtr = sh.tile([P, NT, M], F32R)    # Qd^T (Vd lhsT + CqT source)
    nc.sync.dma_start(qdtr[:], io["QdTr"].rearrange("(t p) m -> p t m", p=P))
    qdt16 = sh.tile([P, NT, M], BF16)   # Qd^T (GT lhsT)
    nc.sync.dma_start(qdt16[:], io["QdT16"].rearrange("(t p) m -> p t m", p=P))
    qdnr = sh.tile([P, MT, N], F32R)    # Qd [m,q] (out lhsT)
    nc.sync.dma_start(qdnr[:], io["QdNr"].rearrange("(c p) q -> p c q", p=P))

    for h in range(HPC):
        # --- per-head input prep ---------------------------------------
        mk = msk_pool.tile([P, NT], F32)
        nc.sync.dma_start(mk[:], io["maskT"][h])

        qt = qt_pool.tile([64, N], F32R)           # Q^T [d,q]
        nc.sync.dma_start(qt[:], io["QT"][h])

        vl = vl_pool.tile([P, NT, D], F32)         # V natural [k,d]
        nc.sync.dma_start(vl[:], io["V"][h].rearrange("(t p) d -> p t d", p=P))
        vm = vm_pool.tile([P, NT, D], F32R)        # (V*mask) rounded to f32r
        for t in range(NT):
            nc.vector.tensor_scalar_mul(vm[:, t, :], vl[:, t, :], mk[:, t : t + 1])

        kl = kl_pool.tile([64, N], F32)            # K^T [d,k] (unmasked)
        nc.sync.dma_start(kl[:], io["KT"][h])
        kts = kt_pool.tile([64, N], F32R)          # (K*mask)^T
        nc.vector.tensor_mul(kts[:], kl[:], maskB[:])

        # --- Vd = Qd @ (V*m)  -> [M, D] --------------------------------
        vd = vd_pool.tile([P, MT, D], F32R)
        for mh in range(MT):
            vps = ps_m.tile([P, D], F32, tag="misc")
            for t in range(NT):
                nc.tensor.matmul(
                    vps[:],
                    lhsT=qdtr[:, t, mh * P : (mh + 1) * P],
                    rhs=vm[:, t, :],
                    start=(t == 0),
                    stop=(t == NT - 1),
                )
            nc.vector.tensor_copy(vd[:, mh, :], vps[:])

        # --- phases A (scores->exp->CqT) and B (A1^T), group-pipelined --
        cq = cq_pool.tile([P, NT, M], BF16)
        a1 = a1_pool.tile([P, NT, NQG, M], BF16)
        exps = {}
        for g in range(NQG):
            # phase A for group g
            for q in range(g * GQ, (g + 1) * GQ):
                ex = exp_pool.tile([P, N], BF16, tag="exp")
                sums = st_pool.tile([P, NCH], F32, tag="sums")
                for c in range(NCH):
                    sps = ps_s.tile([P, SCH], F32, tag="s")
                    for j in range(SCH // 512):
                        nc.tensor.matmul(
                            sps[:, j * 512 : (j + 1) * 512],
                            lhsT=qt[:, q * P : (q + 1) * P],
                            rhs=kts[:, c * SCH + j * 512 : c * SCH + (j + 1) * 512],
                            start=True,
                            stop=True,
                        )
                    nc.scalar.activation(
                        ex[:, c * SCH : (c + 1) * SCH],
                        sps[:],
                        AF.Exp,
                        scale=0.125,
                        accum_out=sums[:, c : c + 1],
                    )
                den = st_pool.tile([P, 1], F32, tag="den")
                if NCH > 1:
                    nc.vector.tensor_reduce(den[:], sums[:], axis=AX.X, op=ALU.add)
                else:
                    den = sums
                rec = st_pool.tile([P, 1], F32, tag="rec")
                nc.vector.reciprocal(rec[:], den[:])
                nc.vector.tensor_scalar_mul(cq[:, q, :], qdtr[:, q, :], rec[:])
                exps[q] = ex

            # phase B for group g: A1^T partial over this group's q-blocks
            for kc in range(NT):
                aps_ = ps_a1.tile([P, M], F32, tag="a1")
                for qi in range(GQ):
                    q = g * GQ + qi
                    nc.tensor.matmul(
                        aps_[:],
                        lhsT=exps[q][:, kc * P : (kc + 1) * P],
                        rhs=cq[:, q, :],
                        start=(qi == 0),
                        stop=(qi == GQ - 1),
                    )
                nc.vector.tensor_copy(a1[:, kc, g, :], aps_[:])

        # --- G^T[n,m] = sum_k QdT[k,n] * A1T[k,m] (both group partials) -
        gt = gt_pool.tile([P, MT, M], F32R)
        gps = ps_gt.tile([P, MT * M], F32, tag="g")
        for nh in range(MT):
            for kc in range(NT):
                for g in range(NQG):
                    nc.tensor.matmul(
                        gps[:, nh * M : (nh + 1) * M],
                        lhsT=qdt16[:, kc, nh * P : (nh + 1) * P],
                        rhs=a1[:, kc, g, :],
                        start=(kc == 0 and g == 0),
                        stop=(kc == NT - 1 and g == NQG - 1),
                    )
            nc.vector.tensor_copy(gt[:, nh, :], gps[:, nh * M : (nh + 1) * M])

        # --- y[m,d] = sum_n GT[n,m] * Vd[n,d] ---------------------------
        yt = y_pool.tile([P, MT, D], F32R)
        for mh in range(MT):
            yps = ps_m.tile([P, D], F32, tag="misc")
            for nh in range(MT):
                nc.tensor.matmul(
                    yps[:],
                    lhsT=gt[:, nh, mh * P : (mh + 1) * P],
                    rhs=vd[:, nh, :],
                    start=(nh == 0),
                    stop=(nh == MT - 1),
                )
            nc.vector.tensor_copy(yt[:, mh, :], yps[:])

        # --- out[q,d] = sum_m Qd[m,q] * y[m,d] --------------------------
        o_r = io["out"][h].rearrange("(t p) d -> t p d", p=P)
        for q in range(NT):
            ops_ = ps_m.tile([P, D], F32, tag="misc")
            for mh in range(MT):
                nc.tensor.matmul(
                    ops_[:],
                    lhsT=qdnr[:, mh, q * P : (q + 1) * P],
                    rhs=yt[:, mh, :],
                    start=(mh == 0),
                    stop=(mh == MT - 1),
                )
            ost = ost_pool.tile([P, D], F32, tag="ost")
            nc.vector.tensor_copy(ost[:], ops_[:])
            nc.sync.dma_start(o_r[q], ost[:])


def build_nc():
    from contextlib import ExitStack

    nc = bacc.Bacc("TRN2", target_bir_lowering=False, debug=False)
    io = {
        "QT": nc.dram_tensor("QT", [HPC, 64, N], F32R, kind="ExternalInput").ap(),
        "KT": nc.dram_tensor("KT", [HPC, 64, N], F32, kind="ExternalInput").ap(),
        "V": nc.dram_tensor("V", [HPC, N, D], F32, kind="ExternalInput").ap(),
        "maskT": nc.dram_tensor("maskT", [HPC, 128, NT], F32, kind="ExternalInput").ap(),
        "maskB": nc.dram_tensor("maskB", [64, N], F32, kind="ExternalInput").ap(),
        "QdTr": nc.dram_tensor("QdTr", [N, M], F32R, kind="ExternalInput").ap(),
        "QdT16": nc.dram_tensor("QdT16", [N, M], BF16, kind="ExternalInput").ap(),
        "QdNr": nc.dram_tensor("QdNr", [M, N], F32R, kind="ExternalInput").ap(),
        "out": nc.dram_tensor("out", [HPC, N, D], F32, kind="ExternalOutput").ap(),
    }
    with tile.TileContext(nc) as tc:
        with ExitStack() as ctx:
            _emit(tc, ctx, io)
    nc.compile()
    return nc


_NC = None


def _get_nc():
    global _NC
    if _NC is None:
        _NC = build_nc()
    return _NC


def make_in_maps(Q, K, V, mask, Q_dct):
    Q = np.asarray(Q, dtype=np.float32).reshape(B * H, N, D)
    K = np.asarray(K, dtype=np.float32).reshape(B * H, N, D)
    V = np.asarray(V, dtype=np.float32).reshape(B * H, N, D)
    mask = np.asarray(mask, dtype=np.float32)
    Q_dct = np.asarray(Q_dct, dtype=np.float32)

    QT = np.ascontiguousarray(Q.transpose(0, 2, 1))
    KT = np.ascontiguousarray(K.transpose(0, 2, 1))
    QdT = np.ascontiguousarray(Q_dct.T)
    QdT16 = QdT.astype(NPBF16)
    QdN = np.ascontiguousarray(Q_dct)
    # maskT[b, p, t] = mask[b, t*128 + p]
    maskT = np.ascontiguousarray(mask.reshape(B, NT, 128).transpose(0, 2, 1))

    in_maps = []
    for c in range(NCORES):
        sl = slice(HPC * c, HPC * (c + 1))
        heads = range(HPC * c, HPC * (c + 1))
        in_maps.append(
            {
                "QT": np.ascontiguousarray(QT[sl]),
                "KT": np.ascontiguousarray(KT[sl]),
                "V": np.ascontiguousarray(V[sl]),
                "maskT": np.ascontiguousarray(
                    np.stack([maskT[hp // H] for hp in heads])
                ),
                "maskB": np.ascontiguousarray(
                    np.broadcast_to(mask[(HPC * c) // H][None, :], (64, N))
                ),
                "QdTr": QdT,
                "QdT16": QdT16,
                "QdNr": QdN,
            }
        )
    return in_maps


def run_on_device(in_maps, **kwargs):
    nc = _get_nc()
    return bass_utils.run_bass_kernel_spmd(
        nc, in_maps, core_ids=list(range(NCORES)), **kwargs
    )


def kernel(Q, K, V, mask, Q_dct):
    in_maps = make_in_maps(Q, K, V, mask, Q_dct)
    res = run_on_device(in_maps)
    out = np.empty((B * H, N, D), dtype=np.float32)
    for c in range(NCORES):
        out[HPC * c : HPC * (c + 1)] = res.results[c]["out"]
    return out.reshape(B, H, N, D)
